# revision 15
# baseline (speedup 1.0000x reference)
"""Classwise-ECE (segmentation) kernel for 8 Trainium2 NeuronCores.

Math: with conf = softmax(logits, axis=C) laid out [C, N], bins
b = ceil(15*conf)-1 in [0,15), the reference ECE reduces to
    sce = mean_c sum_b |D[c,b]| / N,
    D[c,b] = sum_n (conf[c,n] - 1[label_n==c]) * 1[bin(conf[c,n])==b]
because |avg_conf-acc|*count == |conf_sum - acc_sum| per (c,b) bucket.

Sharding: pixels (N = B*H*W) split across 8 cores. Each core computes
partial D histograms [19,15]; host sums them and finalizes the scalar.

Per-core layout: partitions = 6 pixel-slots x 19 classes = 114 rows.
Per 512-pixel chunk: exp on ACT; per-slot softmax denominators via a
block-ones matmul on PE; reciprocal on DVE (packed across chunks);
1/S and labels broadcast back to all 19 class rows via a second
block-ones matmul; conf and v = labeq-conf on DVE; bin index via the
round-to-int magic-bias trick on ACT; then 15 fused compare-multiply-
accumulate (scalar_tensor_tensor) passes on DVE in bf16, one per bin,
accumulating per-(slot,class) sums into an SBUF accumulator.
"""

import numpy as np

C = 19
NB = 15
SLOTS = 6
P = SLOTS * C            # 114 partitions
FD = 512                 # pixels per chunk per slot
B, H, W = 4, 512, 1024
N = B * H * W            # 2097152 pixels
N_CORES = 8
NPC = N // N_CORES       # 262144 pixels per core
CHUNKS = -(-NPC // (SLOTS * FD))   # 86
NF = CHUNKS * FD         # 44032 pixels per slot
NPIX = SLOTS * NF        # 264192 incl. padding
NPAD = NPIX - NPC        # 2048 zero-logit pad pixels per core
GROUP = 3                # chunks per reciprocal batch (32-row spacing)
SROWS = 32 * (GROUP - 1) + SLOTS   # 102 packed S partitions per group
MAGIC = 8388608.0        # 2^23

_CACHE = {}


def _build_program():
    from contextlib import ExitStack
    import concourse.bass as bass
    import concourse.tile as tile
    from concourse import bacc, mybir

    f32 = mybir.dt.float32
    bf16 = mybir.dt.bfloat16
    ALU = mybir.AluOpType
    ACTF = mybir.ActivationFunctionType

    nc = bacc.Bacc("TRN2", target_bir_lowering=False, debug=False,
                   num_devices=N_CORES)

    lg = nc.dram_tensor("lg", [P, NF], f32, kind="ExternalInput").ap()
    lb = nc.dram_tensor("lb", [SLOTS, NF], f32, kind="ExternalInput").ap()
    w1 = nc.dram_tensor("w1", [P, GROUP * SROWS], f32,
                        kind="ExternalInput").ap()
    w2 = nc.dram_tensor("w2", [SROWS, P], f32, kind="ExternalInput").ap()
    cid = nc.dram_tensor("cid", [P, 1], f32, kind="ExternalInput").ap()
    hist = nc.dram_tensor("hist", [P, NB], f32, kind="ExternalOutput").ap()

    with tile.TileContext(nc) as tc, ExitStack() as ctx:
        const_pool = ctx.enter_context(tc.tile_pool(name="const", bufs=1))
        in_pool = ctx.enter_context(tc.tile_pool(name="inp", bufs=3))
        lf_pool = ctx.enter_context(tc.tile_pool(name="lf", bufs=2 * GROUP + 2))
        et_pool = ctx.enter_context(tc.tile_pool(name="et", bufs=2 * GROUP + 2))
        wk_pool = ctx.enter_context(tc.tile_pool(name="wk", bufs=3))
        r_pool = ctx.enter_context(tc.tile_pool(name="rp", bufs=2))
        ps_s = ctx.enter_context(
            tc.tile_pool(name="ps_s", bufs=2, space=bass.MemorySpace.PSUM))
        ps_rb = ctx.enter_context(
            tc.tile_pool(name="ps_rb", bufs=2, space=bass.MemorySpace.PSUM))
        ps_lb = ctx.enter_context(
            tc.tile_pool(name="ps_lb", bufs=2, space=bass.MemorySpace.PSUM))

        w1_sb = const_pool.tile([P, GROUP * SROWS], f32)
        nc.sync.dma_start(w1_sb[:], w1)
        w2_sb = const_pool.tile([SROWS, P], f32)
        nc.sync.dma_start(w2_sb[:], w2)
        cid_sb = const_pool.tile([P, 1], f32)
        nc.sync.dma_start(cid_sb[:], cid)
        negm = const_pool.tile([P, 1], f32)
        nc.gpsimd.memset(negm[:], -MAGIC)
        acc = const_pool.tile([P, NB * CHUNKS], f32)

        ngroups = -(-CHUNKS // GROUP)
        for g in range(ngroups):
            ks = list(range(g * GROUP, min((g + 1) * GROUP, CHUNKS)))
            nk = len(ks)
            spack = ps_s.tile([SROWS, FD], f32, tag="spack")
            ets, lfs = [], []
            for j, k in enumerate(ks):
                lt = in_pool.tile([P, FD], f32, tag="lt")
                nc.sync.dma_start(lt[:], lg[:, k * FD:(k + 1) * FD])
                lfj = lf_pool.tile([SLOTS, FD], f32, tag="lf")
                nc.sync.dma_start(lfj[:], lb[:, k * FD:(k + 1) * FD])
                et = et_pool.tile([P, FD], f32, tag="et")
                nc.scalar.activation(et[:], lt[:], ACTF.Exp)
                # chunk j's per-slot sums land on rows 32j..32j+5 via PSUM
                # accumulation (w1_j is zero elsewhere)
                nc.tensor.matmul(spack[:], w1_sb[:, j * SROWS:(j + 1) * SROWS],
                                 et[:], start=(j == 0), stop=(j == nk - 1))
                ets.append(et)
                lfs.append(lfj)
            rpack = r_pool.tile([SROWS, FD], f32, tag="rpack")
            nc.vector.reciprocal(rpack[:], spack[:])
            for j, k in enumerate(ks):
                rb = ps_rb.tile([P, FD], f32, tag="rb")
                nc.tensor.matmul(rb[:], w2_sb[32 * j:32 * j + SLOTS, :],
                                 rpack[32 * j:32 * j + SLOTS, :],
                                 start=True, stop=True)
                lbb = ps_lb.tile([P, FD], f32, tag="lbb")
                nc.tensor.matmul(lbb[:], w2_sb[0:SLOTS, :], lfs[j][:],
                                 start=True, stop=True)
                cf = wk_pool.tile([P, FD], f32, tag="cf")
                nc.vector.tensor_mul(cf[:], ets[j][:], rb[:])
                vt = wk_pool.tile([P, FD], bf16, tag="vt")
                nc.vector.scalar_tensor_tensor(
                    vt[:], lbb[:], cid_sb[:], cf[:],
                    op0=ALU.is_equal, op1=ALU.subtract)
                y = wk_pool.tile([P, FD], f32, tag="y")
                nc.scalar.activation(y[:], cf[:], ACTF.Copy,
                                     bias=MAGIC - 0.5, scale=15.0)
                bi = wk_pool.tile([P, FD], bf16, tag="bi")
                nc.scalar.activation(bi[:], y[:], ACTF.Relu, bias=negm[:],
                                     scale=1.0)
                trash = wk_pool.tile([P, FD], bf16, tag="trash")
                for t in range(NB):
                    col = t * CHUNKS + k
                    nc.vector.scalar_tensor_tensor(
                        trash[:], bi[:], float(t), vt[:],
                        op0=ALU.is_equal, op1=ALU.mult,
                        accum_out=acc[:, col:col + 1])

        hist_sb = const_pool.tile([P, NB], f32)
        acc3 = acc[:].rearrange("p (t k) -> p t k", k=CHUNKS)
        nc.vector.tensor_reduce(hist_sb[:], acc3, axis=mybir.AxisListType.X,
                                op=mybir.AluOpType.add)
        nc.sync.dma_start(hist, hist_sb[:])

    nc.compile()
    return nc


def _get_program():
    if "nc" not in _CACHE:
        _CACHE["nc"] = _build_program()
    return _CACHE["nc"]


def _host_constants():
    w1 = np.zeros((P, GROUP * SROWS), np.float32)
    w2 = np.zeros((SROWS, P), np.float32)
    cidv = np.zeros((P, 1), np.float32)
    for s in range(SLOTS):
        for c in range(C):
            p = s * C + c
            for j in range(GROUP):
                w1[p, j * SROWS + 32 * j + s] = 1.0
                w2[32 * j + s, p] = 1.0
            cidv[p, 0] = c
    return w1, w2, cidv


def kernel(logits, labels, _trace=False):
    from concourse.bass_utils import run_bass_kernel_spmd

    logits = np.asarray(logits, dtype=np.float32)
    labels = np.asarray(labels)
    lt = np.moveaxis(logits, 1, 0).reshape(C, N)
    lf = labels.reshape(N).astype(np.float32)

    w1, w2, cidv = _host_constants()
    in_maps = []
    for i in range(N_CORES):
        sl = slice(i * NPC, (i + 1) * NPC)
        lgc = np.zeros((C, NPIX), np.float32)
        lgc[:, :NPC] = lt[:, sl]
        lgc = np.ascontiguousarray(
            lgc.reshape(C, SLOTS, NF).transpose(1, 0, 2).reshape(P, NF))
        lbc = np.zeros((NPIX,), np.float32)
        lbc[:NPC] = lf[sl]
        lbc = np.ascontiguousarray(lbc.reshape(SLOTS, NF))
        in_maps.append({"lg": lgc, "lb": lbc, "w1": w1, "w2": w2,
                        "cid": cidv})

    nc = _get_program()
    res = run_bass_kernel_spmd(nc, in_maps, list(range(N_CORES)),
                               trace=_trace)
    _CACHE["last_exec_ns"] = res.exec_time_ns

    hist_agg = np.zeros((P, NB), np.float64)
    for r in res.results:
        hist_agg += r["hist"].astype(np.float64)
    hist_cb = hist_agg.reshape(SLOTS, C, NB).sum(axis=0)   # [19, 15]
    # remove zero-logit padding (label 0, conf 1/19 -> bin 0)
    pad_total = NPAD * N_CORES
    r19 = np.float64(np.float32(1.0) / np.float32(19.0))
    hist_cb[:, 0] -= pad_total * ((np.arange(C) == 0).astype(np.float64) - r19)
    D = -hist_cb
    sce = np.abs(D).sum(axis=1).mean() / N
    return np.float32(sce)
